# revision 1
# baseline (speedup 1.0000x reference)
"""MetacognitionModule (MoE routing) Trainium2 kernel.

Sharding: data-parallel over batch — core i handles batch i (B=8, 8 cores).
Everything is local per core: the router (mean-pool -> 3-layer MLP -> double
softmax) and all 8 expert MLPs run on the core that owns the batch, so no
collectives are needed.

Per-core dataflow (S=2048 tokens, H=2048, Hh=1024, E=8 experts):
  - x[b] is pre-cast to bf16 on host; DMA-transpose loads xT tiles [h,s].
    Main-loop chunk transposes ride the Sync HWDGE queue; the router's
    extra pre-pass transposes (chunks 2,3 only) ride the Scalar HWDGE
    queue so they don't delay the expert pipeline. Chunks 0,1 are pooled
    straight from the main-loop tiles.
  - Router: pooled = mean_s x (DVE free-dim reduces over xT tiles), then
    tiny bf16 matmuls; softmax twice; w broadcast to all partitions via a
    K=1 matmul against a ones column. Only the accumulate-combine ops
    depend on the router, so expert matmuls start immediately.
  - Experts, chunked over S (4 chunks of 512 tokens), expert-inner,
    weights streamed per (chunk, expert):
      L1: heT[f,s] = relu(W1[e].T @ xT + b1)   (bias via ACT per-partition)
      L2: z[s,h]  = heT.T @ W2[e] + ones*b2    (bias via K=1 ones-row matmul)
      acc[s,h]   += w[e] * tanh(z)             (ACT tanh+scale, DVE add)
  - acc chunks stored straight to DRAM in natural [S,H] layout.
All expert matmuls bf16 with fp32 PSUM accumulation.
"""

import sys

for _p in ("/opt/trn_rl_repo", "/root/.axon_site/_ro/trn_rl_repo"):
    if _p not in sys.path:
        sys.path.insert(0, _p)

import ml_dtypes
import numpy as np

import concourse.bacc as bacc
import concourse.bass as bass
import concourse.mybir as mybir
import concourse.tile as tile
from concourse.bass_utils import run_bass_kernel_spmd

BF16 = ml_dtypes.bfloat16
F32 = mybir.dt.float32
BF = mybir.dt.bfloat16
AF = mybir.ActivationFunctionType
ALU = mybir.AluOpType

B, S, H, M, E = 8, 2048, 2048, 256, 8
Hh = H // 2
CHUNK = 512
NCHUNK = S // CHUNK          # 4
NST = CHUNK // 128           # 4 s-subtiles per chunk
NHT = H // 512               # 4 output h tiles (512 wide)
NFT = Hh // 128              # 8 L1 output f tiles
NKH = H // 128               # 16 k tiles over h

_NC = {}


def _softmax_1x8(nc, pool, vec, out, tagp):
    """vec, out: [1, E] f32 sbuf APs. out = softmax(vec) along free dim."""
    mx = pool.tile([1, 1], F32, tag=tagp + "mx", name=tagp + "mx")
    nc.vector.tensor_reduce(mx[:], vec, mybir.AxisListType.X, ALU.max)
    t = pool.tile([1, E], F32, tag=tagp + "t", name=tagp + "t")
    nc.vector.tensor_scalar(t[:], vec, mx[0:1, 0:1], None, ALU.subtract)
    nc.scalar.activation(t[:], t[:], AF.Exp)
    sm = pool.tile([1, 1], F32, tag=tagp + "sm", name=tagp + "sm")
    nc.vector.tensor_reduce(sm[:], t[:], mybir.AxisListType.X, ALU.add)
    rs = pool.tile([1, 1], F32, tag=tagp + "rs", name=tagp + "rs")
    nc.vector.reciprocal(rs[:], sm[:])
    nc.vector.tensor_scalar(out, t[:], rs[0:1, 0:1], None, ALU.mult)


def build(with_bias2=True):
    nc = bacc.Bacc("TRN2", target_bir_lowering=False, debug=False, num_devices=B)

    x_d = nc.dram_tensor("x", [S, H], BF, kind="ExternalInput")
    # W1/W2 arrive host-preshuffled to SBUF layout:
    # W1: [E, 2, 128, 8*Hh]  (halves of the h-contraction, partition-major)
    # W2: [E, 128, NFT*H]    (f-contraction partition-major)
    # W1: [E, half, p, kt, f]  (host-preshuffled, halves of h-contraction)
    # W2: [E, p, ht, fk, c]    (host-preshuffled, ht-major)
    w1_d = nc.dram_tensor("W1", [E, 2, 128, 8, Hh], BF, kind="ExternalInput")
    w2_d = nc.dram_tensor("W2", [E, 128, 4, NFT, 512], BF, kind="ExternalInput")
    b1_d = nc.dram_tensor("b1", [E, Hh], F32, kind="ExternalInput")
    b2_d = nc.dram_tensor("b2", [E, H], BF, kind="ExternalInput")
    wm1_d = nc.dram_tensor("Wm1", [128, NKH * M], BF, kind="ExternalInput")
    bm1_d = nc.dram_tensor("bm1", [M], F32, kind="ExternalInput")
    wm2_d = nc.dram_tensor("Wm2", [128, 2 * M], BF, kind="ExternalInput")
    bm2_d = nc.dram_tensor("bm2", [M], F32, kind="ExternalInput")
    wm3_d = nc.dram_tensor("Wm3", [128, 2 * E], BF, kind="ExternalInput")
    bm3_d = nc.dram_tensor("bm3", [E], F32, kind="ExternalInput")
    eff_d = nc.dram_tensor("eff", [E], F32, kind="ExternalInput")
    out_d = nc.dram_tensor("out", [S, H], F32, kind="ExternalOutput")

    with tile.TileContext(nc) as tc:
        with (
            tc.tile_pool(name="persist", bufs=1) as pp,
            tc.tile_pool(name="router", bufs=1) as rp,
            tc.tile_pool(name="router_xt", bufs=4) as rxp,
            tc.tile_pool(name="xt", bufs=3) as xtp,
            tc.tile_pool(name="w1", bufs=1) as w1p,
            tc.tile_pool(name="w2", bufs=1) as w2p,
            tc.tile_pool(name="bias", bufs=1) as bp,
            tc.tile_pool(name="he", bufs=2) as hep,
            tc.tile_pool(name="acc", bufs=1) as accp,
            tc.tile_pool(name="ye", bufs=4) as yep,
            tc.tile_pool(name="ps1", bufs=2, space=bass.MemorySpace.PSUM) as ps1p,
            tc.tile_pool(name="ps2", bufs=4, space=bass.MemorySpace.PSUM) as ps2p,
            tc.tile_pool(name="rps", bufs=1, space=bass.MemorySpace.PSUM) as rpsp,
        ):
            wbc = pp.tile([128, E], F32)       # router weights, bcast to 128 parts
            ones_bf = pp.tile([1, 128], BF)    # ones row for bias matmuls
            nc.vector.memset(ones_bf[:], 1.0)
            pooled_f = pp.tile([128, NKH], F32)
            nc.vector.memset(pooled_f[:], 0.0)

            def pool_reduce(src, ht, tmp_name):
                r = rxp.tile([128, 1], F32, tag="rred", name=tmp_name)
                nc.vector.tensor_reduce(r[:], src, mybir.AxisListType.X, ALU.add)
                nc.vector.tensor_tensor(
                    pooled_f[:, ht:ht + 1], pooled_f[:, ht:ht + 1], r[:], ALU.add
                )

            # Main-loop xT tiles for chunks 0,1 — hoisted so the router can
            # pool from them before the expert loop starts reading wbc.
            xt_pre = {}
            for ck in (0, 1):
                xt = xtp.tile([128, NKH, CHUNK], BF, tag="xt", name=f"xt{ck}")
                for ht in range(NKH):
                    nc.sync.dma_start_transpose(
                        xt[:, ht, :],
                        x_d[ck * CHUNK:(ck + 1) * CHUNK, ht * 128:(ht + 1) * 128],
                    )
                xt_pre[ck] = xt
            def load_w1(ck, e, split_engines=False):
                w1h = []
                for half in range(2):
                    t = w1p.tile([128, 8, Hh], BF, tag=f"w1h{half}",
                                 name=f"w1_{ck}_{e}_{half}")
                    eng = nc.scalar if (split_engines and half == 1) else nc.gpsimd
                    eng.dma_start(t[:], w1_d[e, half])
                    w1h.append(t)
                return w1h

            def load_w2(ck, e):
                # single contiguous load; ht-major layout keeps L2's first
                # ht section at the front of the transfer
                w2 = w2p.tile([128, 4, NFT, 512], BF, tag="w2", name=f"w2_{ck}_{e}")
                nc.gpsimd.dma_start(w2[:], w2_d[e])
                return w2

            def load_b(ck, e):
                b1t = bp.tile([128, NFT], F32, tag="b1", name=f"b1_{ck}_{e}")
                nc.gpsimd.dma_start(b1t[:], b1_d[e].rearrange("(t p) -> p t", p=128))
                b2t = None
                if with_bias2:
                    b2t = bp.tile([1, H], BF, tag="b2", name=f"b2_{ck}_{e}")
                    nc.gpsimd.dma_start(b2t[:], b2_d[e:e + 1, :])
                return b1t, b2t

            # Expert 0's weights at the head of the SWDGE queue.
            with tc.high_priority():
                preload = {(0, 0): (load_w1(0, 0), load_w2(0, 0), load_b(0, 0))}

            # Chunks 2,3 are pooled from natural-layout x tiles via PE
            # ones-matmuls (pooledT[h] = x_tile.T @ ones), using PE's idle
            # startup window — no extra transposes on the sync queue.
            ones_col = pp.tile([128, 1], BF)
            nc.vector.memset(ones_col[:], 1.0)
            psp = rpsp.tile([128, NKH], F32, tag="rps", name="pool_ps")
            for sk in range(8):          # s-tiles 8..15 = chunks 2,3
                xn = rxp.tile([128, H], BF, tag="xnat", name=f"xnat{sk}")
                nc.scalar.dma_start(
                    xn[:], x_d[1024 + sk * 128:1024 + (sk + 1) * 128, :]
                )
                for ht in range(NKH):
                    nc.tensor.matmul(
                        psp[:, ht:ht + 1],
                        xn[:, ht * 128:(ht + 1) * 128],
                        ones_col[:],
                        start=(sk == 0), stop=(sk == 7),
                        skip_group_check=True,
                    )

            def emit_router_tail():
                """Everything after pooled_f is complete: scale, MLP, softmaxes,
                broadcast of w. Expert matmuls don't depend on any of this."""
                pooled = rp.tile([128, NKH], BF)
                nc.vector.tensor_scalar(pooled[:], pooled_f[:], 1.0 / S, None, ALU.mult)

                wm1 = rp.tile([128, NKH, M], BF)
                nc.gpsimd.dma_start(wm1[:], wm1_d[:].rearrange("p (t f) -> p t f", f=M))
                bm1 = rp.tile([128, 2], F32)
                nc.gpsimd.dma_start(bm1[:], bm1_d[:].rearrange("(t p) -> p t", p=128))
                wm2 = rp.tile([128, 2, M], BF)
                nc.gpsimd.dma_start(wm2[:], wm2_d[:].rearrange("p (t f) -> p t f", f=M))
                bm2 = rp.tile([128, 2], F32)
                nc.gpsimd.dma_start(bm2[:], bm2_d[:].rearrange("(t p) -> p t", p=128))
                wm3 = rp.tile([128, 2, E], BF)
                nc.gpsimd.dma_start(wm3[:], wm3_d[:].rearrange("p (t f) -> p t f", f=E))
                bm3 = rp.tile([1, E], F32)
                nc.gpsimd.dma_start(bm3[:], bm3_d[:].rearrange("(a e) -> a e", a=1))
                eff = rp.tile([1, E], F32)
                nc.gpsimd.dma_start(eff[:], eff_d[:].rearrange("(a e) -> a e", a=1))
                ones_f = rp.tile([1, 128], F32)
                nc.vector.memset(ones_f[:], 1.0)
                ones_b1 = rp.tile([1, 1], BF)
                nc.vector.memset(ones_b1[:], 1.0)

                h1t = rp.tile([128, 2], BF)
                for ft in range(2):
                    ps = rpsp.tile([128, E], F32, tag="rps", name=f"rps1_{ft}")
                    for kt in range(NKH):
                        nc.tensor.matmul(
                            ps[:, 0:1],
                            wm1[:, kt, ft * 128:(ft + 1) * 128],
                            pooled[:, kt:kt + 1],
                            start=(kt == 0), stop=(kt == NKH - 1),
                        )
                    nc.vector.tensor_scalar(
                        h1t[:, ft:ft + 1], ps[:, 0:1], bm1[:, ft:ft + 1], 0.0,
                        ALU.add, ALU.max,
                    )
                h2t = rp.tile([128, 2], BF)
                for ft in range(2):
                    ps = rpsp.tile([128, E], F32, tag="rps", name=f"rps2_{ft}")
                    for kt in range(2):
                        nc.tensor.matmul(
                            ps[:, 0:1],
                            wm2[:, kt, ft * 128:(ft + 1) * 128],
                            h2t_src(h1t, kt),
                            start=(kt == 0), stop=(kt == 1),
                        )
                    nc.vector.tensor_scalar(
                        h2t[:, ft:ft + 1], ps[:, 0:1], bm2[:, ft:ft + 1], 0.0,
                        ALU.add, ALU.max,
                    )
                psl = rpsp.tile([128, E], F32, tag="rps", name="rpsl")
                for kt in range(2):
                    nc.tensor.matmul(
                        psl[0:1, :], h2t[:, kt:kt + 1], wm3[:, kt, :],
                        start=(kt == 0), stop=False,
                    )
                nc.tensor.matmul(
                    psl[0:1, :], ones_b1[0:1, 0:1], bm3_bf(bm3), start=False, stop=True
                )
                logits = rp.tile([1, E], F32)
                nc.vector.tensor_copy(logits[:], psl[0:1, :])

                probs = rp.tile([1, E], F32)
                _softmax_1x8(nc, rp, logits[:], probs[:], "sm1")
                wpre = rp.tile([1, E], F32)
                nc.vector.tensor_tensor(wpre[:], probs[:], eff[:], ALU.mult)
                wrow = rp.tile([1, E], F32)
                _softmax_1x8(nc, rp, wpre[:], wrow[:], "sm2")

                psw = rpsp.tile([128, E], F32, tag="rps", name="rpsw")
                nc.tensor.matmul(psw[:], ones_f[0:1, :], wrow[0:1, :], start=True, stop=True)
                nc.vector.tensor_copy(wbc[:], psw[:])

            def h2t_src(h1t, kt):
                return h1t[:, kt:kt + 1]

            _bm3bf = {}

            def bm3_bf(bm3):
                if "t" not in _bm3bf:
                    t = rp.tile([1, E], BF)
                    nc.vector.tensor_copy(t[:], bm3[:])
                    _bm3bf["t"] = t
                return _bm3bf["t"][0:1, :]

            # Router pooling: chunks 0,1 from the hoisted main tiles, 2,3
            # from the pre-pass tiles; then the full router tail. All before
            # any expert combine reads wbc.
            for ck in (0, 1):
                for ht in range(NKH):
                    pool_reduce(xt_pre[ck][:, ht, :], ht, f"rr{ck}_{ht}")
            nc.vector.tensor_tensor(pooled_f[:], pooled_f[:], psp[:], ALU.add)
            emit_router_tail()

            # ---------------- experts ----------------
            for ck in range(NCHUNK):
                if ck in xt_pre:
                    xt = xt_pre[ck]
                else:
                    xt = xtp.tile([128, NKH, CHUNK], BF, tag="xt", name=f"xt{ck}")
                    for ht in range(NKH):
                        nc.sync.dma_start_transpose(
                            xt[:, ht, :],
                            x_d[ck * CHUNK:(ck + 1) * CHUNK, ht * 128:(ht + 1) * 128],
                        )

                acc_tiles = [
                    accp.tile([128, H], F32, tag=f"acc{st}", name=f"acc{ck}_{st}")
                    for st in range(NST)
                ]
                for e in range(E):
                    if (ck, e) in preload:
                        w1h, w2, (b1t, b2t) = preload[(ck, e)]
                    else:
                        w1h = load_w1(ck, e)
                        w2 = load_w2(ck, e)
                        b1t, b2t = load_b(ck, e)

                    he = hep.tile([128, NFT, CHUNK], BF, tag="he", name=f"he_{ck}_{e}")
                    for ft in range(NFT):
                        ps = ps1p.tile([128, CHUNK], F32, tag="ps1", name=f"ps1_{ck}_{e}_{ft}")
                        for kt in range(NKH):
                            nc.tensor.matmul(
                                ps[:],
                                w1h[kt // 8][:, kt % 8, ft * 128:(ft + 1) * 128],
                                xt[:, kt, :],
                                start=(kt == 0), stop=(kt == NKH - 1),
                            )
                        nc.scalar.activation(
                            he[:, ft, :], ps[:], AF.Relu, bias=b1t[:, ft:ft + 1],
                        )
                    for ht in range(NHT):
                        for st in range(NST):
                            ps2 = ps2p.tile([128, 512], F32, tag="ps2",
                                            name=f"ps2_{ck}_{e}_{st}_{ht}")
                            for fk in range(NFT):
                                nc.tensor.matmul(
                                    ps2[:],
                                    he[:, fk, st * 128:(st + 1) * 128],
                                    w2[:, ht, fk, :],
                                    start=(fk == 0),
                                    stop=(not with_bias2 and fk == NFT - 1),
                                )
                            if with_bias2:
                                nc.tensor.matmul(
                                    ps2[:], ones_bf[0:1, :],
                                    b2t[0:1, ht * 512:(ht + 1) * 512],
                                    start=False, stop=True,
                                )
                            ye = yep.tile([128, 512], F32, tag="ye", name=f"ye_{ck}_{e}_{st}_{ht}")
                            nc.scalar.activation(ye[:], ps2[:], AF.Tanh)
                            accs = acc_tiles[st][:, ht * 512:(ht + 1) * 512]
                            if e == 0:
                                nc.vector.tensor_scalar(
                                    accs, ye[:], wbc[:, 0:1], None, ALU.mult
                                )
                            else:
                                nc.vector.scalar_tensor_tensor(
                                    accs, ye[:], wbc[:, e:e + 1], accs,
                                    ALU.mult, ALU.add,
                                )
                    if e == E - 1:
                        for st in range(NST):
                            r0 = ck * CHUNK + st * 128
                            nc.gpsimd.dma_start(out_d[r0:r0 + 128, :], acc_tiles[st][:])

    nc.compile()
    return nc


def _get_nc(with_bias2=True):
    if with_bias2 not in _NC:
        _NC[with_bias2] = build(with_bias2)
    return _NC[with_bias2]


def prep_in_maps(inputs):
    x = np.asarray(inputs["x"], np.float32)
    xbf = x.astype(BF16)
    w1 = np.asarray(inputs["W1"], np.float32).astype(BF16)   # [E, H, Hh]
    w2 = np.asarray(inputs["W2"], np.float32).astype(BF16)   # [E, Hh, H]
    # shuffle to SBUF layout (see build()): halves x partition-major
    w1s = np.ascontiguousarray(
        w1.reshape(E, 2, 8, 128, Hh).transpose(0, 1, 3, 2, 4)
    )
    w2s = np.ascontiguousarray(
        w2.reshape(E, 8, 128, 4, 512).transpose(0, 2, 3, 1, 4)
    )
    wm1 = np.asarray(inputs["Wm1"], np.float32).astype(BF16)
    wm1s = np.ascontiguousarray(
        wm1.reshape(16, 128, M).transpose(1, 0, 2).reshape(128, 16 * M)
    )
    wm2 = np.asarray(inputs["Wm2"], np.float32).astype(BF16)
    wm2s = np.ascontiguousarray(
        wm2.reshape(2, 128, M).transpose(1, 0, 2).reshape(128, 2 * M)
    )
    wm3 = np.asarray(inputs["Wm3"], np.float32).astype(BF16)
    wm3s = np.ascontiguousarray(
        wm3.reshape(2, 128, E).transpose(1, 0, 2).reshape(128, 2 * E)
    )
    shared = {
        "W1": w1s,
        "W2": w2s,
        "b1": np.asarray(inputs["b1"], np.float32),
        "b2": np.asarray(inputs["b2"], np.float32).astype(BF16),
        "Wm1": wm1s,
        "bm1": np.asarray(inputs["bm1"], np.float32),
        "Wm2": wm2s,
        "bm2": np.asarray(inputs["bm2"], np.float32),
        "Wm3": wm3s,
        "bm3": np.asarray(inputs["bm3"], np.float32),
        "eff": np.asarray(inputs["eff"], np.float32),
    }
    return [dict(shared, x=xbf[b]) for b in range(B)]


def kernel(**inputs):
    wb2 = bool(np.any(np.asarray(inputs["b2"])))
    nc = _get_nc(wb2)
    in_maps = prep_in_maps(inputs)
    res = run_bass_kernel_spmd(nc, in_maps, core_ids=list(range(B)))
    return np.stack([r["out"] for r in res.results])


if __name__ == "__main__":
    rng = np.random.default_rng(0)
    s = 0.02
    ins = {
        "x": rng.standard_normal((B, S, H), dtype=np.float32),
        "Wm1": rng.standard_normal((H, M), dtype=np.float32) * s,
        "bm1": np.zeros(M, np.float32),
        "Wm2": rng.standard_normal((M, M), dtype=np.float32) * s,
        "bm2": np.zeros(M, np.float32),
        "Wm3": rng.standard_normal((M, E), dtype=np.float32) * s,
        "bm3": np.zeros(E, np.float32),
        "W1": rng.standard_normal((E, H, Hh), dtype=np.float32) * s,
        "b1": np.zeros((E, Hh), np.float32),
        "W2": rng.standard_normal((E, Hh, H), dtype=np.float32) * s,
        "b2": np.zeros((E, H), np.float32),
        "eff": np.ones(E, np.float32),
    }
    out = kernel(**ins)
    print("out", out.shape, out.dtype, float(np.abs(out).mean()))



# revision 2
# speedup vs baseline: 1.0719x; 1.0719x over previous
"""MetacognitionModule (MoE routing) Trainium2 kernel — bf16 v2.

Sharding: data-parallel over batch — core i handles batch i (B=8, 8 cores).
All 8 expert MLPs + the router run locally per core; no collectives.

Differences vs the v1 baseline (which idled the PE ~124us: a ~47us router
stall on the pooling pre-pass + ~70us of weight-DMA starvation in chunk 0):
  - x arrives host-pretransposed as one resident SBUF tile [128, 16kt, 2048s]
    (64KB/part): no DMA-transposes, no natural-layout router pre-pass.
  - Router pooling is pure DVE (one reduce per chunk off the resident x),
    emitted after expert 0's L1 so nothing blocks the expert pipeline; the
    router tail's tiny matmuls slot between L1_e0 and L2_e0 on the PE.
  - W1 double-buffered (bufs=2) and split across both DMA queues for the
    first expert; W2 streamed as two h-halves, single-buffered ping-pong.
    Steady-state weight DMA (8MB/expert over 2 queues) hides under the
    ~27us of PE work per expert.
  - acc + out in fp16 (half the combine-DVE and store-DMA cost); host
    upcasts. Adds ~0.1% rel err on top of bf16's 0.34% — tolerance is 2e-2.

Per (chunk of 512 tokens) x expert:
  L1: he[f,s] = relu(W1[e].T @ xT + b1), 8 psum tiles x 16 matmuls
  L2: z[s,h] = he.T @ W2[e], 16 psum tiles x 8 matmuls
  combine: acc[s,h] += w_e * tanh(z)   (ACT tanh -> fp16, DVE fma)
All matmuls bf16 with fp32 PSUM accumulation.
"""

import sys

for _p in ("/opt/trn_rl_repo", "/root/.axon_site/_ro/trn_rl_repo"):
    if _p not in sys.path:
        sys.path.insert(0, _p)

import ml_dtypes
import numpy as np

import concourse.bacc as bacc
import concourse.bass as bass
import concourse.mybir as mybir
import concourse.tile as tile
from concourse.bass_utils import run_bass_kernel_spmd

BF16 = ml_dtypes.bfloat16
F32 = mybir.dt.float32
F16 = mybir.dt.float16
BF = mybir.dt.bfloat16
AF = mybir.ActivationFunctionType
ALU = mybir.AluOpType

B, S, H, M, E = 8, 2048, 2048, 256, 8
Hh = H // 2
CHUNK = 512
NCHUNK = S // CHUNK          # 4
NST = CHUNK // 128           # 4 s-subtiles per chunk
NHT = H // 512               # 4 L2 output h tiles (512 wide)
NFT = Hh // 128              # 8 L1 output f tiles
NKH = H // 128               # 16 k tiles over h
NFK = Hh // 128              # 8 k tiles over f (L2 contraction)

_NC = {}


def _softmax_1x8(nc, pool, vec, out, tagp):
    """vec, out: [1, E] f32 sbuf APs. out = softmax(vec) along free dim."""
    mx = pool.tile([1, 1], F32, tag=tagp + "mx", name=tagp + "mx")
    nc.vector.tensor_reduce(mx[:], vec, mybir.AxisListType.X, ALU.max)
    t = pool.tile([1, E], F32, tag=tagp + "t", name=tagp + "t")
    nc.vector.tensor_scalar(t[:], vec, mx[0:1, 0:1], None, ALU.subtract)
    nc.scalar.activation(t[:], t[:], AF.Exp)
    sm = pool.tile([1, 1], F32, tag=tagp + "sm", name=tagp + "sm")
    nc.vector.tensor_reduce(sm[:], t[:], mybir.AxisListType.X, ALU.add)
    rs = pool.tile([1, 1], F32, tag=tagp + "rs", name=tagp + "rs")
    nc.vector.reciprocal(rs[:], sm[:])
    nc.vector.tensor_scalar(out, t[:], rs[0:1, 0:1], None, ALU.mult)


def build(with_bias2=False):
    nc = bacc.Bacc("TRN2", target_bir_lowering=False, debug=False, num_devices=B)

    x_d = nc.dram_tensor("x", [NCHUNK, 128, NKH, CHUNK], BF, kind="ExternalInput")
    w1_d = nc.dram_tensor("W1", [E, 2, 128, NKH, Hh // 2], BF, kind="ExternalInput")
    w2_d = nc.dram_tensor("W2", [E, 2, 128, NFK, H // 2], BF, kind="ExternalInput")
    b1_d = nc.dram_tensor("b1", [E, 128, NFT], F32, kind="ExternalInput")
    b2_d = nc.dram_tensor("b2", [E, H], BF, kind="ExternalInput")
    wm1_d = nc.dram_tensor("Wm1", [128, NKH * M], BF, kind="ExternalInput")
    bm1_d = nc.dram_tensor("bm1", [M], F32, kind="ExternalInput")
    wm2_d = nc.dram_tensor("Wm2", [128, 2 * M], BF, kind="ExternalInput")
    bm2_d = nc.dram_tensor("bm2", [M], F32, kind="ExternalInput")
    wm3_d = nc.dram_tensor("Wm3", [128, 2 * E], BF, kind="ExternalInput")
    bm3_d = nc.dram_tensor("bm3", [E], F32, kind="ExternalInput")
    eff_d = nc.dram_tensor("eff", [E], F32, kind="ExternalInput")
    out_d = nc.dram_tensor("out", [S, H], F16, kind="ExternalOutput")

    with tile.TileContext(nc) as tc:
        with (
            tc.tile_pool(name="persist", bufs=1) as pp,
            tc.tile_pool(name="router", bufs=1) as rp,
            tc.tile_pool(name="w1a", bufs=2) as w1ap,
            tc.tile_pool(name="w1b", bufs=2) as w1bp,
            tc.tile_pool(name="w2a", bufs=1) as w2ap,
            tc.tile_pool(name="w2b", bufs=1) as w2bp,
            tc.tile_pool(name="bias", bufs=2) as bp,
            tc.tile_pool(name="he", bufs=2) as hep,
            tc.tile_pool(name="acc", bufs=1) as accp,
            tc.tile_pool(name="ye", bufs=4) as yep,
            tc.tile_pool(name="ps1", bufs=2, space=bass.MemorySpace.PSUM) as ps1p,
            tc.tile_pool(name="ps2", bufs=5, space=bass.MemorySpace.PSUM) as ps2p,
            tc.tile_pool(name="rps", bufs=1, space=bass.MemorySpace.PSUM) as rpsp,
        ):
            wbc = pp.tile([128, E], F32)       # router weights, bcast to 128 parts
            pooled_f = pp.tile([128, NKH], F32)

            # ---- resident xT (all chunks, 64KB/part) ----
            xt = pp.tile([128, NCHUNK, NKH, CHUNK], BF)
            for ck in range(NCHUNK):
                nc.sync.dma_start(xt[:, ck], x_d[ck])

            def load_w(ck, e, split00=False):
                # W1 halves (k-tiles 0-7 / 8-15); both needed by every L1 tile.
                w1a = w1ap.tile([128, NKH, Hh // 2], BF, tag="w1a", name=f"w1a_{ck}_{e}")
                w1b = w1bp.tile([128, NKH, Hh // 2], BF, tag="w1b", name=f"w1b_{ck}_{e}")
                if split00:
                    # first expert: spread W1 over both queues to start PE
                    # early; only W1 jumps the queues — W2 isn't needed until
                    # ~70us in and must not preempt the x loads on sync
                    with tc.high_priority():
                        nc.gpsimd.dma_start(w1a[:], w1_d[e, 0])
                        nc.scalar.dma_start(w1b[:], w1_d[e, 1])
                else:
                    nc.gpsimd.dma_start(w1a[:], w1_d[e, 0])
                    nc.gpsimd.dma_start(w1b[:], w1_d[e, 1])
                # W2 h-halves (ht 0-1 / 2-3), single-buffered ping-pong. Their
                # WAR-blocked triggers live on the sync queue (idle mid-chunk)
                # so they never stall the ACT stream on Scalar.
                w2a = w2ap.tile([128, NFK, H // 2], BF, tag="w2a", name=f"w2a_{ck}_{e}")
                nc.sync.dma_start(w2a[:], w2_d[e, 0])
                w2b = w2bp.tile([128, NFK, H // 2], BF, tag="w2b", name=f"w2b_{ck}_{e}")
                nc.sync.dma_start(w2b[:], w2_d[e, 1])
                b1t = bp.tile([128, NFT], F32, tag="b1", name=f"b1_{ck}_{e}")
                nc.gpsimd.dma_start(b1t[:], b1_d[e])
                b2t = None
                if with_bias2:
                    b2t = bp.tile([1, H], BF, tag="b2", name=f"b2_{ck}_{e}")
                    nc.gpsimd.dma_start(b2t[:], b2_d[e:e + 1, :])
                return w1a, w1b, w2a, w2b, b1t, b2t

            preload = {(0, 0): load_w(0, 0, split00=True)}
            # expert 1's weights queued ahead of the router weights
            preload[(0, 1)] = load_w(0, 1)

            # router weight loads issued early (gpsimd queue, behind (0,0) W1)
            wm1 = rp.tile([128, NKH, M], BF)
            nc.gpsimd.dma_start(wm1[:], wm1_d[:].rearrange("p (t f) -> p t f", f=M))
            bm1 = rp.tile([128, 2], F32)
            nc.gpsimd.dma_start(bm1[:], bm1_d[:].rearrange("(t p) -> p t", p=128))
            wm2 = rp.tile([128, 2, M], BF)
            nc.gpsimd.dma_start(wm2[:], wm2_d[:].rearrange("p (t f) -> p t f", f=M))
            bm2 = rp.tile([128, 2], F32)
            nc.gpsimd.dma_start(bm2[:], bm2_d[:].rearrange("(t p) -> p t", p=128))
            wm3 = rp.tile([128, 2, E], BF)
            nc.gpsimd.dma_start(wm3[:], wm3_d[:].rearrange("p (t f) -> p t f", f=E))
            bm3 = rp.tile([1, E], F32)
            nc.gpsimd.dma_start(bm3[:], bm3_d[:].rearrange("(a e) -> a e", a=1))
            eff = rp.tile([1, E], F32)
            nc.gpsimd.dma_start(eff[:], eff_d[:].rearrange("(a e) -> a e", a=1))
            ones_bf = pp.tile([1, 128], BF)    # ones row for b2 matmul (if used)
            nc.vector.memset(ones_bf[:], 1.0)

            def emit_router():
                """Pool from resident xT (DVE), then MLP -> softmax^2 -> wbc."""
                pr = [
                    rp.tile([128, NKH], F32, tag=f"pr{c}", name=f"pool_red{c}")
                    for c in range(NCHUNK)
                ]
                for c in range(NCHUNK):
                    nc.vector.tensor_reduce(
                        pr[c][:], xt[:, c], mybir.AxisListType.X, ALU.add
                    )
                nc.vector.tensor_tensor(pr[0][:], pr[0][:], pr[1][:], ALU.add)
                nc.vector.tensor_tensor(pr[2][:], pr[2][:], pr[3][:], ALU.add)
                nc.vector.tensor_tensor(pooled_f[:], pr[0][:], pr[2][:], ALU.add)
                pooled = rp.tile([128, NKH], BF)
                nc.vector.tensor_scalar(pooled[:], pooled_f[:], 1.0 / S, None, ALU.mult)

                ones_f = rp.tile([1, 128], F32)
                nc.vector.memset(ones_f[:], 1.0)
                ones_b1 = rp.tile([1, 1], BF)
                nc.vector.memset(ones_b1[:], 1.0)
                bm3b = rp.tile([1, E], BF)
                nc.vector.tensor_copy(bm3b[:], bm3[:])

                h1t = rp.tile([128, 2], BF)
                for ft in range(2):
                    ps = rpsp.tile([128, E], F32, tag="rps", name=f"rps1_{ft}")
                    for kt in range(NKH):
                        nc.tensor.matmul(
                            ps[:, 0:1],
                            wm1[:, kt, ft * 128:(ft + 1) * 128],
                            pooled[:, kt:kt + 1],
                            start=(kt == 0), stop=(kt == NKH - 1),
                        )
                    nc.vector.tensor_scalar(
                        h1t[:, ft:ft + 1], ps[:, 0:1], bm1[:, ft:ft + 1], 0.0,
                        ALU.add, ALU.max,
                    )
                h2t = rp.tile([128, 2], BF)
                for ft in range(2):
                    ps = rpsp.tile([128, E], F32, tag="rps", name=f"rps2_{ft}")
                    for kt in range(2):
                        nc.tensor.matmul(
                            ps[:, 0:1],
                            wm2[:, kt, ft * 128:(ft + 1) * 128],
                            h1t[:, kt:kt + 1],
                            start=(kt == 0), stop=(kt == 1),
                        )
                    nc.vector.tensor_scalar(
                        h2t[:, ft:ft + 1], ps[:, 0:1], bm2[:, ft:ft + 1], 0.0,
                        ALU.add, ALU.max,
                    )
                psl = rpsp.tile([128, E], F32, tag="rps", name="rpsl")
                for kt in range(2):
                    nc.tensor.matmul(
                        psl[0:1, :], h2t[:, kt:kt + 1], wm3[:, kt, :],
                        start=(kt == 0), stop=False,
                    )
                nc.tensor.matmul(
                    psl[0:1, :], ones_b1[0:1, 0:1], bm3b[0:1, :], start=False, stop=True
                )
                logits = rp.tile([1, E], F32)
                nc.vector.tensor_copy(logits[:], psl[0:1, :])

                probs = rp.tile([1, E], F32)
                _softmax_1x8(nc, rp, logits[:], probs[:], "sm1")
                wpre = rp.tile([1, E], F32)
                nc.vector.tensor_tensor(wpre[:], probs[:], eff[:], ALU.mult)
                wrow = rp.tile([1, E], F32)
                _softmax_1x8(nc, rp, wpre[:], wrow[:], "sm2")

                psw = rpsp.tile([128, E], F32, tag="rps", name="rpsw")
                nc.tensor.matmul(psw[:], ones_f[0:1, :], wrow[0:1, :],
                                 start=True, stop=True)
                nc.vector.tensor_copy(wbc[:], psw[:])

            # ---------------- experts ----------------
            wtiles = {}
            hetiles = {}

            def l1(ck, e):
                if (ck, e) in preload:
                    wtiles[e] = preload[(ck, e)]
                else:
                    wtiles[e] = load_w(ck, e)
                w1a, w1b, _, _, b1t, _ = wtiles[e]
                he = hep.tile([128, NFK, CHUNK], BF, tag="he", name=f"he_{ck}_{e}")
                hetiles[e] = he
                for ft in range(NFT):
                    ps = ps1p.tile([128, CHUNK], F32, tag="ps1",
                                   name=f"ps1_{ck}_{e}_{ft}")
                    fsl = slice((ft % 4) * 128, (ft % 4 + 1) * 128)
                    w1h = (w1a, w1b)[ft // 4]
                    for kt in range(NKH):
                        nc.tensor.matmul(
                            ps[:],
                            w1h[:, kt, fsl],
                            xt[:, ck, kt],
                            start=(kt == 0), stop=(kt == NKH - 1),
                        )
                    nc.scalar.activation(
                        he[:, ft, :], ps[:], AF.Relu, bias=b1t[:, ft:ft + 1],
                    )

            def l2(ck, e, acc_tiles):
                _, _, w2a, w2b, _, b2t = wtiles.pop(e)
                he = hetiles.pop(e)
                # last expert runs st-outer and stores each acc piece right
                # after its combine, so the output DMA overlaps the rest
                last = e == E - 1
                hs_order = (
                    [(ht, st) for st in range(NST) for ht in range(NHT)]
                    if last else
                    [(ht, st) for ht in range(NHT) for st in range(NST)]
                )
                for ht, st in hs_order:
                    w2h = (w2a, w2b)[ht // 2]
                    hsl2 = slice((ht % 2) * 512, (ht % 2 + 1) * 512)
                    ps2 = ps2p.tile([128, 512], F32, tag="ps2",
                                    name=f"ps2_{ck}_{e}_{st}_{ht}")
                    ssl = slice(st * 128, (st + 1) * 128)
                    for fk in range(NFK):
                        nc.tensor.matmul(
                            ps2[:],
                            he[:, fk, ssl],
                            w2h[:, fk, hsl2],
                            start=(fk == 0),
                            stop=(not with_bias2 and fk == NFK - 1),
                        )
                    if with_bias2:
                        nc.tensor.matmul(
                            ps2[:], ones_bf[0:1, :],
                            b2t[0:1, ht * 512:(ht + 1) * 512],
                            start=False, stop=True,
                        )
                    ye = yep.tile([128, 512], F16, tag="ye",
                                  name=f"ye_{ck}_{e}_{st}_{ht}")
                    nc.scalar.activation(ye[:], ps2[:], AF.Tanh)
                    accs = acc_tiles[st][:, ht * 512:(ht + 1) * 512]
                    if e == 0:
                        nc.vector.tensor_scalar(
                            accs, ye[:], wbc[:, 0:1], None, ALU.mult
                        )
                    else:
                        nc.vector.scalar_tensor_tensor(
                            accs, ye[:], wbc[:, e:e + 1], accs,
                            ALU.mult, ALU.add,
                        )
                    if last:
                        r0 = ck * CHUNK + st * 128
                        hw = 512 * ht
                        nc.sync.dma_start(
                            out_d[r0:r0 + 128, hw:hw + 512],
                            acc_tiles[st][:, hw:hw + 512],
                        )

            for ck in range(NCHUNK):
                acc_tiles = [
                    accp.tile([128, H], F16, tag=f"acc{st}", name=f"acc{ck}_{st}")
                    for st in range(NST)
                ]
                if ck == 0:
                    # offset pipeline: two L1s before the first L2 defers the
                    # first combine (and thus the pooling/router deadline)
                    # past the HBM-saturated startup window
                    l1(0, 0)
                    l1(0, 1)
                    emit_router()
                    for e in range(E):
                        l2(0, e, acc_tiles)
                        if e + 2 < E:
                            l1(0, e + 2)
                else:
                    for e in range(E):
                        l1(ck, e)
                        l2(ck, e, acc_tiles)

    nc.compile()
    return nc


def _get_nc(with_bias2=False):
    if with_bias2 not in _NC:
        _NC[with_bias2] = build(with_bias2)
    return _NC[with_bias2]


def prep_in_maps(inputs):
    x = np.asarray(inputs["x"], np.float32).astype(BF16)       # [B, S, H]
    # -> [B, ck, 128p, kt, 512s]; h = kt*128 + p
    xs = np.ascontiguousarray(
        x.reshape(B, NCHUNK, CHUNK, NKH, 128).transpose(0, 1, 4, 3, 2)
    )

    w1 = np.asarray(inputs["W1"], np.float32).astype(BF16)     # [E, H, Hh]
    # -> [E, 2(f-half), 128p, 16kt, 512f]
    w1s = np.ascontiguousarray(
        w1.reshape(E, NKH, 128, 2, Hh // 2).transpose(0, 3, 2, 1, 4)
    )
    w2 = np.asarray(inputs["W2"], np.float32).astype(BF16)     # [E, Hh, H]
    # -> [E, 2(h-half), 128p, 8fk, 1024h]
    w2s = np.ascontiguousarray(
        w2.reshape(E, NFK, 128, 2, H // 2).transpose(0, 3, 2, 1, 4)
    )
    b1 = np.ascontiguousarray(
        np.asarray(inputs["b1"], np.float32).reshape(E, NFT, 128).transpose(0, 2, 1)
    )

    wm1 = np.asarray(inputs["Wm1"], np.float32).astype(BF16)
    wm1s = np.ascontiguousarray(
        wm1.reshape(NKH, 128, M).transpose(1, 0, 2).reshape(128, NKH * M)
    )
    wm2 = np.asarray(inputs["Wm2"], np.float32).astype(BF16)
    wm2s = np.ascontiguousarray(
        wm2.reshape(2, 128, M).transpose(1, 0, 2).reshape(128, 2 * M)
    )
    wm3 = np.asarray(inputs["Wm3"], np.float32).astype(BF16)
    wm3s = np.ascontiguousarray(
        wm3.reshape(2, 128, E).transpose(1, 0, 2).reshape(128, 2 * E)
    )
    shared = {
        "W1": w1s,
        "W2": w2s,
        "b1": b1,
        "b2": np.asarray(inputs["b2"], np.float32).astype(BF16),
        "Wm1": wm1s,
        "bm1": np.asarray(inputs["bm1"], np.float32),
        "Wm2": wm2s,
        "bm2": np.asarray(inputs["bm2"], np.float32),
        "Wm3": wm3s,
        "bm3": np.asarray(inputs["bm3"], np.float32),
        "eff": np.asarray(inputs["eff"], np.float32),
    }
    return [dict(shared, x=xs[b]) for b in range(B)]


def kernel(**inputs):
    wb2 = bool(np.any(np.asarray(inputs["b2"])))
    nc = _get_nc(wb2)
    in_maps = prep_in_maps(inputs)
    res = run_bass_kernel_spmd(nc, in_maps, core_ids=list(range(B)))
    return np.stack([r["out"].astype(np.float32) for r in res.results])


if __name__ == "__main__":
    rng = np.random.default_rng(0)
    s = 0.02
    ins = {
        "x": rng.standard_normal((B, S, H), dtype=np.float32),
        "Wm1": rng.standard_normal((H, M), dtype=np.float32) * s,
        "bm1": np.zeros(M, np.float32),
        "Wm2": rng.standard_normal((M, M), dtype=np.float32) * s,
        "bm2": np.zeros(M, np.float32),
        "Wm3": rng.standard_normal((M, E), dtype=np.float32) * s,
        "bm3": np.zeros(E, np.float32),
        "W1": rng.standard_normal((E, H, Hh), dtype=np.float32) * s,
        "b1": np.zeros((E, Hh), np.float32),
        "W2": rng.standard_normal((E, Hh, H), dtype=np.float32) * s,
        "b2": np.zeros((E, H), np.float32),
        "eff": np.ones(E, np.float32),
    }
    out = kernel(**ins)
    print("out", out.shape, out.dtype, float(np.abs(out).mean()))
